# revision 1
# baseline (speedup 1.0000x reference)
"""Trainium2 Bass kernel: CodebookWrapperLinear (vq-codebook quantized linear).

Computes out[b,s,o] = sum_i x[b,s,i] * w[o,i] where
  w[o, g*GS+j] = (codebook / max|codebook|)[indexes[o,g,j]] * exp(scale[o,g])

v2 changes vs v1 (both 8-way tensor-parallel over out-features):
  - weights live in 8 SBUF tiles keyed by (n-block, k-half) so Tile's
    whole-tile dependency tracking lets matmuls start as soon as their own
    slice of the dequant is done (v1 serialized the full 725us prep before
    the first matmul).
  - dequant is bf16 end-to-end (DVE 16-bit path), idx loaded via HWDGE so
    the Pool/SWDGE queue only carries x casting DMAs.
  - GEMM runs in three waves so prep overlaps compute:
      early wave   m in [0, SPLIT):   n-blocks 0..1 (first half of weights);
                                      xT also cached to DRAM
      full wave    m in [SPLIT, MT):  all 4 n-blocks
      cleanup wave m in [0, SPLIT):   n-blocks 2..3, xT read back from DRAM
    Prep dequantizes n-blocks in order 0,1,2,3 so later blocks hide under
    the early/full waves.
"""

import math

import numpy as np

B, S, IN, OUT, GS = 4, 2048, 4096, 16384, 32
G = IN // GS  # 128
N_CORES = 8
N_SHARD = OUT // N_CORES  # 2048

_BUILD_CACHE = {}


def _fit_cubic(codebook):
    """Exact cubic through (t, cb_norm[i]) for t = i - 1.5, i = 0..3."""
    cb = np.asarray(codebook, np.float64).reshape(-1)
    assert cb.shape == (4,), cb.shape
    cbn = cb / np.clip(np.abs(cb).max(), 1e-8, None)
    t = np.array([-1.5, -0.5, 0.5, 1.5])
    V = np.vander(t, 4, increasing=True)  # [1, t, t^2, t^3]
    c = np.linalg.solve(V, cbn)
    return tuple(float(v) for v in c)


def _build(M, N, K, coefs, n_cores):
    """Emit the Bass program: out[M,N] = x[M,K] @ dequant(idx[N,K], scl[N,G]).T"""
    from concourse import bacc
    import concourse.bass as bass
    import concourse.mybir as mybir
    from concourse.tile import TileContext

    f32 = mybir.dt.float32
    bf16 = mybir.dt.bfloat16
    i32 = mybir.dt.int32
    AOT = mybir.AluOpType
    AFT = mybir.ActivationFunctionType

    c0, c1, c2, c3 = coefs
    antisym = abs(c0) < 1e-9 and abs(c2) < 1e-9 and c3 > 1e-12

    Gn = K // GS  # groups per out row (128)
    OC = N // 128  # out chunks (16)
    MT = M // 128  # token tiles (64)
    KC = K // 128  # k chunks (32)
    NBW = 512  # n-block width (one PSUM bank)
    NB = N // NBW  # 4
    OCB = NBW // 128  # out chunks per n-block (4)
    SUB = 1024  # dequant subtile width
    SUBC = SUB // 128  # 8 j-blocks per transpose
    GSUB = SUB // GS  # groups per subtile (32)
    NSUB = K // SUB  # subtiles per out-chunk (4)
    KHC = KC // 2  # k-chunks per k-half (16)
    SPLIT = 6  # early/cleanup wave m-tiles
    assert MT > SPLIT and NB == 4 and NSUB == 4

    nc = bacc.Bacc(
        "TRN2", target_bir_lowering=False, debug=False, num_devices=n_cores
    )
    x = nc.dram_tensor("x", [M, K], f32, kind="ExternalInput")
    idx = nc.dram_tensor("idx", [N, K], i32, kind="ExternalInput")
    scl = nc.dram_tensor("scl", [N, Gn], f32, kind="ExternalInput")
    out = nc.dram_tensor("out", [M, N], f32, kind="ExternalOutput")

    with TileContext(nc, num_cores=n_cores) as tc:
        with tc.tile_pool(name="wt", bufs=1) as wt_pool, tc.tile_pool(
            name="es", bufs=1
        ) as es_pool, tc.tile_pool(name="xtd", bufs=1, space="DRAM") as dram_pool, tc.tile_pool(
            name="mm", bufs=1
        ) as mp, tc.tile_pool(name="ps", bufs=1, space="PSUM") as psp:
            # weight tiles: [n-block][k-half] each [128, KHC*NBW] bf16
            #   col = j_local*NBW + (oc%OCB)*128 + o_local
            wt = [
                [
                    wt_pool.tile(
                        [128, KHC * NBW], bf16, name=f"wt{n}_{kh}", tag=f"wt{n}_{kh}"
                    )
                    for kh in range(2)
                ]
                for n in range(NB)
            ]
            es_t = [
                es_pool.tile([128, Gn], bf16, name=f"es{oc}", tag=f"es{oc}")
                for oc in range(OC)
            ]
            xtd = dram_pool.tile([128, SPLIT * K], bf16, name="xtd")
            ebias_t = es_pool.tile([128, 1], f32, name="ebias")
            es_bias = math.log(c3) if antisym else 0.0
            nc.vector.memset(ebias_t[:, :], es_bias)
            tbias_t = es_pool.tile([128, 1], f32, name="tbias")
            nc.vector.memset(tbias_t[:, :], -1.5)

            # ---------- GEMM helpers ----------
            def x_tile(m, cache):
                xt = mp.tile([128, K], bf16, tag="xt", bufs=2, name="xt")
                xt_r = xt[:, :].rearrange("p (j t) -> p j t", t=128)
                for h in range(2):
                    xb = mp.tile([128, K // 2], bf16, tag="xb", bufs=2, name="xb")
                    # SWDGE casting DMA: f32 DRAM -> bf16 SBUF
                    nc.gpsimd.dma_start(
                        xb[:, :],
                        x[
                            m * 128 : (m + 1) * 128,
                            h * (K // 2) : (h + 1) * (K // 2),
                        ],
                    )
                    nc.sync.dma_start_transpose(
                        xt_r[:, h * KHC : (h + 1) * KHC, :], xb[:, :]
                    )
                if cache:
                    nc.sync.dma_start(xtd[:, m * K : (m + 1) * K], xt[:, :])
                return xt

            def mm_block(m, xt, n):
                ps = psp.tile([128, NBW], f32, tag="ps", bufs=8, name="ps")
                for j in range(KC):
                    kh, jl = divmod(j, KHC)
                    nc.tensor.matmul(
                        ps[:, :],
                        xt[:, j * 128 : (j + 1) * 128],
                        wt[n][kh][:, jl * NBW : (jl + 1) * NBW],
                        start=(j == 0),
                        stop=(j == KC - 1),
                    )
                ob = mp.tile([128, NBW], f32, tag="ob", bufs=4, name="ob")
                nc.any.tensor_copy(ob[:, :], ps[:, :])
                nc.sync.dma_start(
                    out[m * 128 : (m + 1) * 128, n * NBW : (n + 1) * NBW],
                    ob[:, :],
                )

            # hoist the first x tiles so their DMAs head the queues
            xt_cache = {m: x_tile(m, cache=True) for m in range(2)}

            # ---------- prep: dequant n-block-major, k-half-major ----------
            with tc.tile_pool(name="prep", bufs=1) as pp:
                for n in range(NB):
                    for oc in range(n * OCB, (n + 1) * OCB):
                        sclt = pp.tile(
                            [128, Gn], f32, tag="sclt", bufs=2, name="sclt"
                        )
                        nc.sync.dma_start(
                            sclt[:, :], scl[oc * 128 : (oc + 1) * 128, :]
                        )
                        nc.scalar.activation(
                            es_t[oc][:, :],
                            sclt[:, :],
                            AFT.Exp,
                            bias=ebias_t[:, :],
                            scale=1.0,
                        )
                    for kh in range(2):
                        for oc in range(n * OCB, (n + 1) * OCB):
                            for s_i in range(2 * kh, 2 * kh + 2):
                                ks = s_i * SUB
                                idxt = pp.tile(
                                    [128, SUB], bf16, tag="idxt", bufs=5, name="idxt"
                                )
                                nc.gpsimd.dma_start(
                                    idxt[:, :],
                                    idx[oc * 128 : (oc + 1) * 128, ks : ks + SUB],
                                )
                                tf = pp.tile(
                                    [128, SUB], bf16, tag="dq", bufs=8, name="tf"
                                )
                                sq = pp.tile(
                                    [128, SUB], bf16, tag="dq", bufs=8, name="sq"
                                )
                                v = pp.tile(
                                    [128, SUB], bf16, tag="dq", bufs=8, name="v"
                                )
                                if antisym:
                                    # all-bf16 DVE chain (16-bit 2x path):
                                    # t = u-1.5; v = (t^2 + c1/c3)*t
                                    nc.vector.tensor_scalar(
                                        tf[:, :], idxt[:, :], 1.5, None, AOT.subtract
                                    )
                                    nc.vector.tensor_mul(
                                        sq[:, :], tf[:, :], tf[:, :]
                                    )
                                    nc.vector.scalar_tensor_tensor(
                                        v[:, :], sq[:, :], c1 / c3, tf[:, :],
                                        AOT.add, AOT.mult,
                                    )
                                elif abs(c3) > 1e-12:
                                    nc.vector.tensor_scalar(
                                        tf[:, :], idxt[:, :], 1.5, None, AOT.subtract
                                    )
                                    nc.vector.tensor_mul(
                                        sq[:, :], tf[:, :], tf[:, :]
                                    )
                                    p = pp.tile(
                                        [128, SUB], bf16, tag="dq", bufs=8, name="p"
                                    )
                                    nc.vector.scalar_tensor_tensor(
                                        p[:, :], sq[:, :], c1 / c3, tf[:, :],
                                        AOT.add, AOT.mult,
                                    )
                                    qv = pp.tile(
                                        [128, SUB], bf16, tag="dq", bufs=8, name="qv"
                                    )
                                    nc.vector.tensor_scalar(
                                        qv[:, :], sq[:, :], c2, c0, AOT.mult, AOT.add
                                    )
                                    nc.vector.scalar_tensor_tensor(
                                        v[:, :], p[:, :], c3, qv[:, :],
                                        AOT.mult, AOT.add,
                                    )
                                else:
                                    nc.vector.tensor_scalar(
                                        tf[:, :], idxt[:, :], 1.5, None, AOT.subtract
                                    )
                                    nc.vector.tensor_mul(
                                        sq[:, :], tf[:, :], tf[:, :]
                                    )
                                    qv = pp.tile(
                                        [128, SUB], bf16, tag="dq", bufs=8, name="qv"
                                    )
                                    nc.vector.tensor_scalar(
                                        qv[:, :], sq[:, :], c2, c0, AOT.mult, AOT.add
                                    )
                                    nc.vector.scalar_tensor_tensor(
                                        v[:, :], tf[:, :], c1, qv[:, :],
                                        AOT.mult, AOT.add,
                                    )
                                wb = pp.tile(
                                    [128, SUB], bf16, tag="wb", bufs=6, name="wb"
                                )
                                g0 = ks // GS
                                es_sl = es_t[oc][:, g0 : g0 + GSUB]
                                v3 = v[:, :].rearrange("p (g s) -> p g s", s=GS)
                                w3 = wb[:, :].rearrange("p (g s) -> p g s", s=GS)
                                es3 = es_sl.rearrange("p (g s) -> p g s", s=1)
                                es3b, _ = bass.broadcast_tensor_aps(es3, v3)
                                nc.vector.tensor_tensor(w3, v3, es3b, AOT.mult)
                                # xbar transpose into wt[n][kh]:
                                #   dest[p, jl, (oc%OCB)*128+o] = wb[o, dj*128+p]
                                jl0 = s_i * SUBC - kh * KHC
                                c0_ = (oc % OCB) * 128
                                wtt = wt[n][kh][:, :].rearrange(
                                    "p (j c) -> p j c", c=NBW
                                )
                                nc.sync.dma_start_transpose(
                                    wtt[:, jl0 : jl0 + SUBC, c0_ : c0_ + 128],
                                    wb[:, :],
                                )

            # ---------- main GEMM: early / full / cleanup waves ----------
            for m in range(SPLIT):  # early wave: out-half 0 only
                xt = xt_cache.pop(m) if m in xt_cache else x_tile(m, cache=True)
                for n in range(NB // 2):
                    mm_block(m, xt, n)
            for m in range(SPLIT, MT):  # full wave
                xt = x_tile(m, cache=False)
                for n in range(NB):
                    mm_block(m, xt, n)
            for m in range(SPLIT):  # cleanup wave: out-half 1
                xt = mp.tile([128, K], bf16, tag="xt", bufs=2, name="xt2")
                nc.sync.dma_start(xt[:, :], xtd[:, m * K : (m + 1) * K])
                for n in range(NB // 2, NB):
                    mm_block(m, xt, n)

    nc.finalize()
    return nc


def get_nc(M, N, K, coefs, n_cores):
    key = (M, N, K, coefs, n_cores)
    if key not in _BUILD_CACHE:
        _BUILD_CACHE[key] = _build(M, N, K, coefs, n_cores)
    return _BUILD_CACHE[key]


def kernel(x, codebook, scale, indexes):
    from concourse import bass_utils

    x = np.asarray(x, dtype=np.float32)
    codebook = np.asarray(codebook, dtype=np.float32)
    scale = np.asarray(scale, dtype=np.float32)
    indexes = np.asarray(indexes, dtype=np.int32)

    Bx, Sx, INx = x.shape
    OUTx = indexes.shape[0]
    M = Bx * Sx
    coefs = _fit_cubic(codebook)

    xm = np.ascontiguousarray(x.reshape(M, INx))
    idx2 = np.ascontiguousarray(indexes.reshape(OUTx, INx))
    scl2 = np.ascontiguousarray(scale.reshape(OUTx, INx // GS))

    n_shard = OUTx // N_CORES
    nc = get_nc(M, n_shard, INx, coefs, N_CORES)

    in_maps = []
    for c in range(N_CORES):
        in_maps.append(
            {
                "x": xm,
                "idx": idx2[c * n_shard : (c + 1) * n_shard],
                "scl": scl2[c * n_shard : (c + 1) * n_shard],
            }
        )
    res = bass_utils.run_bass_kernel_spmd(
        nc, in_maps, core_ids=list(range(N_CORES))
    )
    out = np.concatenate(
        [res.results[c]["out"] for c in range(N_CORES)], axis=1
    )
    return out.reshape(Bx, Sx, OUTx)



# revision 2
# speedup vs baseline: 2.1860x; 2.1860x over previous
"""Trainium2 Bass kernel: CodebookWrapperLinear (vq-codebook quantized linear).

Computes out[b,s,o] = sum_i x[b,s,i] * w[o,i] where
  w[o, g*GS+j] = (codebook / max|codebook|)[indexes[o,g,j]] * exp(scale[o,g])

8-way tensor-parallel over out-features; per-core GEMM is
  out_T[N_shard, M] = dequant(idx, scl) @ xT[K, M].

v5 = v4 + hybrid-precision k-split: the last 2*KF k-chunks run as KF fp8e4
DoubleRow matmuls (2x PE rate, contracting two 128-k planes each).  The fp8
weight slice is dequantized in bf16 with its group scale premultiplied by KW,
then cast to fp8; the matching x slice arrives from host as fp8(x/KW), so the
scaling cancels exactly inside each MAC.  KW centers exp(scale) in the e4m3
binade.  Measured (deterministic inputs): rel_err ~0.0155 @ KF=3, ~0.0178 @
KF=4 vs the 2e-2 gate; fp8 DoubleRow microbenches at 1.98x bf16.

Layout/encoding prep on host (no dequant arithmetic beyond dtype casts): xT
transposed + bf16; idx shipped as t = idx - 1.5 bf16; x8 = fp8(xT/KW) slice;
out returns transposed, reassembled on host.

Engines: ACT sq=Square(t) + exps; DVE cubic + 1/4 of group-scale mults +
psum flushes; Pool 3/4 of group-scale mults + wst->fp8 casts; SP transposes,
x/idx/scl loads, out writes.  GEMM: mc0 oc-major (tracks dequant), mc>=1
j-major in two passes of 8 psum banks.
"""

import math

import numpy as np

B, S, IN, OUT, GS = 4, 2048, 4096, 16384, 32
G = IN // GS  # 128
N_CORES = 8
N_SHARD = OUT // N_CORES  # 2048

KF = 4  # fp8 k-chunk pairs per matmul tile (2*KF k-chunks of 128)
KW = 13.0  # fp8 weight prescale (cancelled by x8 = fp8(x/KW))

_BUILD_CACHE = {}


def _fit_cubic(codebook):
    """Exact cubic through (t, cb_norm[i]) for t = i - 1.5, i = 0..3."""
    cb = np.asarray(codebook, np.float64).reshape(-1)
    assert cb.shape == (4,), cb.shape
    cbn = cb / np.clip(np.abs(cb).max(), 1e-8, None)
    t = np.array([-1.5, -0.5, 0.5, 1.5])
    V = np.vander(t, 4, increasing=True)  # [1, t, t^2, t^3]
    c = np.linalg.solve(V, cbn)
    return tuple(float(v) for v in c)


def _build(M, N, K, coefs, n_cores, kf=KF, kw=KW):
    from concourse import bacc
    import concourse.bass as bass
    import concourse.mybir as mybir
    from concourse.tile import TileContext

    f32 = mybir.dt.float32
    bf16 = mybir.dt.bfloat16
    fp8 = mybir.dt.float8e4
    AOT = mybir.AluOpType
    AFT = mybir.ActivationFunctionType
    PM = mybir.MatmulPerfMode

    c0, c1, c2, c3 = coefs
    antisym = abs(c0) < 1e-9 and abs(c2) < 1e-9 and c3 > 1e-12
    assert antisym, "hybrid path assumes antisymmetric codebook"
    # antisym codebook is piecewise-linear in t: cbn = A*t + B*sgn(t), exact
    # through (+-0.5, +-1.5).  A folds into the exp bias; stt applies B/A.
    alpha = c1 * 0.5 + c3 * 0.125  # cbn at t=0.5
    beta = c1 * 1.5 + c3 * 3.375  # cbn at t=1.5
    A = beta - alpha
    BoA = (3 * alpha - beta) / (2 * A)

    Gn = K // GS  # groups per out row (128)
    OC = N // 128  # out chunks (16)
    MC = M // 512  # m chunks (16)
    KC = K // 128  # k chunks (32)
    JS = KC - 2 * kf  # bf16 k-chunks
    KS = JS * 128  # k split point
    GSPLIT = KS // GS  # first fp8 group
    SUB = 1024  # dequant subtile width (k)
    SUBC = SUB // 128  # k-chunks per dequant subtile (8)
    GSUB = SUB // GS  # groups per subtile (32)
    NSUB = K // SUB  # dequant subtiles per out-chunk (4)
    XJ = 4 if JS % 4 == 0 else 2  # bf16 k-chunks per x tile
    XT_N = JS // XJ  # x tiles per mc
    XB = {4: 9, 2: 19}[XJ]  # x tile bufs (1.5 mc lookahead)

    nc = bacc.Bacc(
        "TRN2", target_bir_lowering=False, debug=False, num_devices=n_cores
    )
    xt_d = nc.dram_tensor("xt", [KS, M], bf16, kind="ExternalInput")
    x8_d = nc.dram_tensor("x8", [2 * kf * 128, M], fp8, kind="ExternalInput")
    idx = nc.dram_tensor("idx", [N, K], bf16, kind="ExternalInput")  # holds t
    scl = nc.dram_tensor("scl", [N, Gn], f32, kind="ExternalInput")
    out = nc.dram_tensor("out", [N, M], f32, kind="ExternalOutput")

    with TileContext(nc, num_cores=n_cores) as tc:
        with tc.tile_pool(name="wt", bufs=1) as wt_pool, tc.tile_pool(
            name="mm", bufs=1
        ) as mp, tc.tile_pool(name="ps", bufs=1, space="PSUM") as psp:
            # per-oc weight tiles: bf16 [128, (j, o)] and fp8 [128, (pair, plane, o)]
            wt = [
                wt_pool.tile([128, JS * 128], bf16, name=f"wt{oc}", tag=f"wt{oc}")
                for oc in range(OC)
            ]
            wt8 = [
                wt_pool.tile(
                    [128, 2 * kf * 128], fp8, name=f"wt8_{oc}", tag=f"wt8_{oc}"
                )
                for oc in range(OC)
            ]
            es_t = [
                wt_pool.tile([128, Gn], bf16, name=f"es{oc}", tag=f"es{oc}")
                for oc in range(OC)
            ]
            ebias_t = wt_pool.tile([128, 1], f32, name="ebias")
            nc.vector.memset(ebias_t[:, :], math.log(A))
            ebias8_t = wt_pool.tile([128, 1], f32, name="ebias8")
            nc.vector.memset(ebias8_t[:, :], math.log(A) + math.log(kw))

            # ---------- prep: dequant, oc-major so oc0 is ready first ------
            with tc.tile_pool(name="prep", bufs=1) as pp:

                def emit_exps(ocs):
                    for oc in ocs:
                        sclt = pp.tile(
                            [128, Gn], f32, tag="sclt", bufs=2, name="sclt"
                        )
                        nc.gpsimd.dma_start(
                            sclt[:, :], scl[oc * 128 : (oc + 1) * 128, :]
                        )
                        nc.scalar.activation(
                            es_t[oc][:, :GSPLIT],
                            sclt[:, :GSPLIT],
                            AFT.Exp,
                            bias=ebias_t[:, :],
                            scale=1.0,
                        )
                        nc.scalar.activation(
                            es_t[oc][:, GSPLIT:],
                            sclt[:, GSPLIT:],
                            AFT.Exp,
                            bias=ebias8_t[:, :],
                            scale=1.0,
                        )

                def dequant_oc(oc):
                    tts = []
                    for h in range(2):
                        tt = pp.tile(
                            [128, K // 2], bf16, tag="tt", bufs=3, name="tt"
                        )
                        nc.gpsimd.dma_start(
                            tt[:, :],
                            idx[
                                oc * 128 : (oc + 1) * 128,
                                h * (K // 2) : (h + 1) * (K // 2),
                            ],
                        )
                        tts.append(tt)
                    wst = pp.tile(
                        [128, 2 * kf * 128], bf16, tag="wst", bufs=2, name="wst"
                    )
                    for s_i in range(NSUB):
                        tsl = tts[s_i // 2][
                            :, (s_i % 2) * SUB : (s_i % 2 + 1) * SUB
                        ]
                        sg = pp.tile([128, SUB], bf16, tag="sg", bufs=3, name="sg")
                        nc.scalar.activation(sg[:, :], tsl, AFT.Sign)
                        v = pp.tile([128, SUB], bf16, tag="v", bufs=3, name="v")
                        # v = t + (B/A)*sgn(t)   (es carries the A)
                        nc.vector.scalar_tensor_tensor(
                            v[:, :], sg[:, :], BoA, tsl, AOT.mult, AOT.add
                        )
                        wb = pp.tile([128, SUB], bf16, tag="wb", bufs=4, name="wb")
                        g0 = (s_i * SUB) // GS
                        es_sl = es_t[oc][:, g0 : g0 + GSUB]
                        v3 = v[:, :].rearrange("p (g s) -> p g s", s=GS)
                        w3 = wb[:, :].rearrange("p (g s) -> p g s", s=GS)
                        es3 = es_sl.rearrange("p (g s) -> p g s", s=1)
                        es3b, _ = bass.broadcast_tensor_aps(es3, v3)
                        # broadcast mult runs 1x everywhere: spread DVE/Pool
                        if s_i == 0:
                            nc.vector.tensor_tensor(w3, v3, es3b, AOT.mult)
                        else:
                            nc.gpsimd.tensor_tensor(w3, v3, es3b, AOT.mult)
                        # transpose: chunks < JS -> wt[oc]; rest -> wst
                        j0 = s_i * SUBC
                        n_bf = min(max(JS - j0, 0), SUBC)
                        if n_bf > 0:
                            wtt = wt[oc][:, :].rearrange("p (j c) -> p j c", c=128)
                            nc.sync.dma_start_transpose(
                                wtt[:, j0 : j0 + n_bf, :], wb[:, : n_bf * 128]
                            )
                        if n_bf < SUBC:
                            wsv = wst[:, :].rearrange("p (j c) -> p j c", c=128)
                            d0 = j0 + n_bf - JS
                            nc.sync.dma_start_transpose(
                                wsv[:, d0 : d0 + SUBC - n_bf, :],
                                wb[:, n_bf * 128 :],
                            )
                    # cast the fp8 slice (Pool has slack)
                    nc.gpsimd.tensor_copy(wt8[oc][:, :], wst[:, :])

                emit_exps(range(2))
                dequant_oc(0)
                emit_exps(range(2, OC))
                for oc in range(1, OC):
                    dequant_oc(oc)

            # ---------- GEMM ----------
            xt_r = xt_d.rearrange("(a b p) m -> p a b m", p=128, b=XJ)
            x8_r = x8_d.rearrange("(pr pl p) m -> p pr pl m", p=128, pl=2)
            wto = [
                wt[oc][:, :].rearrange("p (j c) -> p j c", c=128) for oc in range(OC)
            ]
            wt8o = [
                wt8[oc][:, :].rearrange("p (pr pl c) -> p pr pl c", pl=2, c=128)
                for oc in range(OC)
            ]

            def load_x(mc):
                xts = []
                for a in range(XT_N):
                    xtile = mp.tile(
                        [128, XJ, 512], bf16, tag="xt", bufs=XB, name="xt"
                    )
                    nc.sync.dma_start(
                        xtile[:, :, :],
                        xt_r[:, a, :, mc * 512 : (mc + 1) * 512],
                    )
                    xts.append(xtile)
                x8t = mp.tile([128, kf, 2, 512], fp8, tag="x8t", bufs=3, name="x8t")
                nc.sync.dma_start(
                    x8t[:, :, :, :], x8_r[:, :, :, mc * 512 : (mc + 1) * 512]
                )
                return xts, x8t

            def mm_tile(ps, xts, x8t, oc):
                for j in range(JS):
                    nc.tensor.matmul(
                        ps[:, :],
                        wto[oc][:, j, :],
                        xts[j // XJ][:, j % XJ, :],
                        start=(j == 0),
                        stop=False,
                    )
                for pr in range(kf):
                    nc.tensor.matmul(
                        ps[:, :],
                        wt8o[oc][:, pr],
                        x8t[:, pr],
                        start=False,
                        stop=(pr == kf - 1),
                        perf_mode=PM.DoubleRow,
                    )

            def flush(ps, oc, mc):
                ob = mp.tile([128, 512], f32, tag="ob", bufs=3, name="ob")
                nc.vector.tensor_copy(ob[:, :], ps[:, :])
                nc.sync.dma_start(
                    out[oc * 128 : (oc + 1) * 128, mc * 512 : (mc + 1) * 512],
                    ob[:, :],
                )

            # mc0: oc-major
            xts, x8t = load_x(0)
            for oc in range(OC):
                ps = psp.tile([128, 512], f32, tag="ps", bufs=8, name="ps")
                mm_tile(ps, xts, x8t, oc)
                flush(ps, oc, 0)

            # mc1..: j-major, two passes of 8 out-chunks
            for mc in range(1, MC):
                xts, x8t = load_x(mc)
                for half in range(2):
                    pss = [
                        psp.tile([128, 512], f32, tag="ps", bufs=8, name="ps")
                        for _ in range(8)
                    ]
                    ocs = list(range(half * 8, half * 8 + 8))
                    for j in range(JS):
                        for i, oc in enumerate(ocs):
                            nc.tensor.matmul(
                                pss[i][:, :],
                                wto[oc][:, j, :],
                                xts[j // XJ][:, j % XJ, :],
                                start=(j == 0),
                                stop=False,
                            )
                    for pr in range(kf):
                        for i, oc in enumerate(ocs):
                            nc.tensor.matmul(
                                pss[i][:, :],
                                wt8o[oc][:, pr],
                                x8t[:, pr],
                                start=False,
                                stop=(pr == kf - 1),
                                perf_mode=PM.DoubleRow,
                            )
                    for i, oc in enumerate(ocs):
                        flush(pss[i], oc, mc)

    nc.finalize()
    return nc


def get_nc(M, N, K, coefs, n_cores):
    key = (M, N, K, coefs, n_cores, KF, KW)
    if key not in _BUILD_CACHE:
        _BUILD_CACHE[key] = _build(M, N, K, coefs, n_cores)
    return _BUILD_CACHE[key]


def make_in_maps(x, codebook, scale, indexes):
    import ml_dtypes

    x = np.asarray(x, dtype=np.float32)
    codebook = np.asarray(codebook, dtype=np.float32)
    scale = np.asarray(scale, dtype=np.float32)
    indexes = np.asarray(indexes, dtype=np.int32)

    Bx, Sx, INx = x.shape
    OUTx = indexes.shape[0]
    M = Bx * Sx
    coefs = _fit_cubic(codebook)
    KS = INx - 2 * KF * 128

    xT = x.reshape(M, INx).T  # [K, M] view
    xt_bf = np.ascontiguousarray(xT[:KS].astype(ml_dtypes.bfloat16))
    x8 = np.ascontiguousarray(
        (xT[KS:] * np.float32(1.0 / KW)).astype(ml_dtypes.float8_e4m3)
    )
    tenc = (indexes.reshape(OUTx, INx).astype(np.float32) - 1.5).astype(
        ml_dtypes.bfloat16
    )
    scl2 = np.ascontiguousarray(scale.reshape(OUTx, INx // GS))

    n_shard = OUTx // N_CORES
    in_maps = []
    for c in range(N_CORES):
        in_maps.append(
            {
                "xt": xt_bf,
                "x8": x8,
                "idx": np.ascontiguousarray(tenc[c * n_shard : (c + 1) * n_shard]),
                "scl": scl2[c * n_shard : (c + 1) * n_shard],
            }
        )
    return in_maps, coefs, (Bx, Sx, INx, OUTx, M, n_shard)


def kernel(x, codebook, scale, indexes):
    from concourse import bass_utils

    in_maps, coefs, (Bx, Sx, INx, OUTx, M, n_shard) = make_in_maps(
        x, codebook, scale, indexes
    )
    nc = get_nc(M, n_shard, INx, coefs, N_CORES)
    res = bass_utils.run_bass_kernel_spmd(nc, in_maps, core_ids=list(range(N_CORES)))
    out_t = np.concatenate(
        [res.results[c]["out"] for c in range(N_CORES)], axis=0
    )  # [OUT, M]
    return np.ascontiguousarray(out_t.T).reshape(Bx, Sx, OUTx)


# revision 4
# speedup vs baseline: 2.3673x; 1.0829x over previous
"""Trainium2 Bass kernel v5: CodebookWrapperLinear (vq-codebook quantized linear).

Computes out[b,s,o] = sum_i x[b,s,i] * w[o,i] where
  w[o, g*GS+j] = (codebook / max|codebook|)[indexes[o,g,j]] * exp(scale[o,g])

8-way tensor-parallel over out-features; per-core GEMM is
  out_T[N_shard, M] = dequant(idx, scl) @ xT[K, M].

v5 = v4 + hybrid-precision k-split: the last 2*KF k-chunks run as KF fp8e4
DoubleRow matmuls (2x PE rate, contracting two 128-k planes each).  The fp8
weight slice is dequantized in bf16 with its group scale premultiplied by KW,
then cast to fp8; the matching x slice arrives from host as fp8(x/KW), so the
scaling cancels exactly inside each MAC.  KW centers exp(scale) in the e4m3
binade.  Measured (deterministic inputs): rel_err ~0.0155 @ KF=3, ~0.0178 @
KF=4 vs the 2e-2 gate; fp8 DoubleRow microbenches at 1.98x bf16.

Layout/encoding prep on host (no dequant arithmetic beyond dtype casts): xT
transposed + bf16; idx shipped as t = idx - 1.5 bf16; x8 = fp8(xT/KW) slice;
out returns transposed, reassembled on host.

Engines: ACT sq=Square(t) + exps; DVE cubic + 1/4 of group-scale mults +
psum flushes; Pool 3/4 of group-scale mults + wst->fp8 casts; SP transposes,
x/idx/scl loads, out writes.  GEMM: mc0 oc-major (tracks dequant), mc>=1
j-major in two passes of 8 psum banks.
"""

import math

import numpy as np

B, S, IN, OUT, GS = 4, 2048, 4096, 16384, 32
G = IN // GS  # 128
N_CORES = 8
N_SHARD = OUT // N_CORES  # 2048

KF = 4  # fp8 k-chunk pairs per matmul tile (2*KF k-chunks of 128)
KW = 13.0  # fp8 weight prescale (cancelled by x8 = fp8(x/KW))

_BUILD_CACHE = {}


def _fit_cubic(codebook):
    """Exact cubic through (t, cb_norm[i]) for t = i - 1.5, i = 0..3."""
    cb = np.asarray(codebook, np.float64).reshape(-1)
    assert cb.shape == (4,), cb.shape
    cbn = cb / np.clip(np.abs(cb).max(), 1e-8, None)
    t = np.array([-1.5, -0.5, 0.5, 1.5])
    V = np.vander(t, 4, increasing=True)  # [1, t, t^2, t^3]
    c = np.linalg.solve(V, cbn)
    return tuple(float(v) for v in c)


def _build(M, N, K, coefs, n_cores, kf=KF, kw=KW):
    from concourse import bacc
    import concourse.bass as bass
    import concourse.mybir as mybir
    from concourse.tile import TileContext

    f32 = mybir.dt.float32
    bf16 = mybir.dt.bfloat16
    fp8 = mybir.dt.float8e4
    AOT = mybir.AluOpType
    AFT = mybir.ActivationFunctionType
    PM = mybir.MatmulPerfMode

    c0, c1, c2, c3 = coefs
    antisym = abs(c0) < 1e-9 and abs(c2) < 1e-9 and c3 > 1e-12
    assert antisym, "hybrid path assumes antisymmetric codebook"
    # antisym codebook is piecewise-linear in t: cbn = A*t + B*sgn(t), exact
    # through (+-0.5, +-1.5).  A folds into the exp bias; stt applies B/A.
    alpha = c1 * 0.5 + c3 * 0.125  # cbn at t=0.5
    beta = c1 * 1.5 + c3 * 3.375  # cbn at t=1.5
    A = beta - alpha
    BoA = (3 * alpha - beta) / (2 * A)

    Gn = K // GS  # groups per out row (128)
    OC = N // 128  # out chunks (16)
    MC = M // 512  # m chunks (16)
    KC = K // 128  # k chunks (32)
    JS = KC - 2 * kf  # bf16 k-chunks
    KS = JS * 128  # k split point
    GSPLIT = KS // GS  # first fp8 group
    SUB = 1024  # dequant subtile width (k)
    SUBC = SUB // 128  # k-chunks per dequant subtile (8)
    GSUB = SUB // GS  # groups per subtile (32)
    NSUB = K // SUB  # dequant subtiles per out-chunk (4)
    XJ = 4 if JS % 4 == 0 else 2  # bf16 k-chunks per x tile
    XT_N = JS // XJ  # x tiles per mc
    XB = {4: 12, 2: 25}[XJ]  # x tile bufs (1.5 mc lookahead)

    nc = bacc.Bacc(
        "TRN2", target_bir_lowering=False, debug=False, num_devices=n_cores
    )
    xt_d = nc.dram_tensor("xt", [KS, M], bf16, kind="ExternalInput")
    x8_d = nc.dram_tensor("x8", [2 * kf * 128, M], fp8, kind="ExternalInput")
    idx = nc.dram_tensor("idx", [N, K], bf16, kind="ExternalInput")  # holds t
    scl = nc.dram_tensor("scl", [N, Gn], f32, kind="ExternalInput")
    out = nc.dram_tensor("out", [N, M], f32, kind="ExternalOutput")

    with TileContext(nc, num_cores=n_cores) as tc:
        with tc.tile_pool(name="wt", bufs=1) as wt_pool, tc.tile_pool(
            name="mm", bufs=1
        ) as mp, tc.tile_pool(name="ps", bufs=1, space="PSUM") as psp:
            # per-oc weight tiles: bf16 [128, (j, o)] and fp8 [128, (pair, plane, o)]
            wt = [
                wt_pool.tile([128, JS * 128], bf16, name=f"wt{oc}", tag=f"wt{oc}")
                for oc in range(OC)
            ]
            wt8 = [
                wt_pool.tile(
                    [128, 2 * kf * 128], fp8, name=f"wt8_{oc}", tag=f"wt8_{oc}"
                )
                for oc in range(OC)
            ]
            es_t = [
                wt_pool.tile([128, Gn], bf16, name=f"es{oc}", tag=f"es{oc}")
                for oc in range(OC)
            ]
            ebias_t = wt_pool.tile([128, 1], f32, name="ebias")
            nc.vector.memset(ebias_t[:, :], math.log(A))
            ebias8_t = wt_pool.tile([128, 1], f32, name="ebias8")
            nc.vector.memset(ebias8_t[:, :], math.log(A) + math.log(kw))

            # ---------- prep: dequant, oc-major so oc0 is ready first ------
            with tc.tile_pool(name="prep", bufs=1) as pp:

                def emit_exps(ocs):
                    for oc in ocs:
                        sclt = pp.tile(
                            [128, Gn], f32, tag="sclt", bufs=1, name="sclt"
                        )
                        nc.gpsimd.dma_start(
                            sclt[:, :], scl[oc * 128 : (oc + 1) * 128, :]
                        )
                        nc.scalar.activation(
                            es_t[oc][:, :GSPLIT],
                            sclt[:, :GSPLIT],
                            AFT.Exp,
                            bias=ebias_t[:, :],
                            scale=1.0,
                        )
                        nc.scalar.activation(
                            es_t[oc][:, GSPLIT:],
                            sclt[:, GSPLIT:],
                            AFT.Exp,
                            bias=ebias8_t[:, :],
                            scale=1.0,
                        )

                def dequant_oc(oc):
                    tts = []
                    for h in range(2):
                        tt = pp.tile(
                            [128, K // 2], bf16, tag="tt", bufs=2, name="tt"
                        )
                        nc.gpsimd.dma_start(
                            tt[:, :],
                            idx[
                                oc * 128 : (oc + 1) * 128,
                                h * (K // 2) : (h + 1) * (K // 2),
                            ],
                        )
                        tts.append(tt)
                    wst = pp.tile(
                        [128, 2 * kf * 128], bf16, tag="wst", bufs=2, name="wst"
                    )
                    for s_i in range(NSUB):
                        tsl = tts[s_i // 2][
                            :, (s_i % 2) * SUB : (s_i % 2 + 1) * SUB
                        ]
                        sg = pp.tile([128, SUB], bf16, tag="sg", bufs=2, name="sg")
                        # input is t' = t - B/A; b = (t' >= 0); v = t' + 2(B/A)b
                        nc.vector.tensor_scalar(
                            sg[:, :], tsl, 0.0, None, AOT.is_ge
                        )
                        v = pp.tile([128, SUB], bf16, tag="v", bufs=2, name="v")
                        nc.vector.scalar_tensor_tensor(
                            v[:, :], sg[:, :], 2 * BoA, tsl, AOT.mult, AOT.add
                        )
                        wb = pp.tile([128, SUB], bf16, tag="wb", bufs=4, name="wb")
                        g0 = (s_i * SUB) // GS
                        es_sl = es_t[oc][:, g0 : g0 + GSUB]
                        v3 = v[:, :].rearrange("p (g s) -> p g s", s=GS)
                        w3 = wb[:, :].rearrange("p (g s) -> p g s", s=GS)
                        es3 = es_sl.rearrange("p (g s) -> p g s", s=1)
                        es3b, _ = bass.broadcast_tensor_aps(es3, v3)
                        # broadcast mult runs 1x everywhere: spread DVE/Pool
                        if s_i == 0:
                            nc.vector.tensor_tensor(w3, v3, es3b, AOT.mult)
                        else:
                            nc.gpsimd.tensor_tensor(w3, v3, es3b, AOT.mult)
                        # transpose: chunks < JS -> wt[oc]; rest -> wst
                        j0 = s_i * SUBC
                        n_bf = min(max(JS - j0, 0), SUBC)
                        if n_bf > 0:
                            wtt = wt[oc][:, :].rearrange("p (j c) -> p j c", c=128)
                            nc.sync.dma_start_transpose(
                                wtt[:, j0 : j0 + n_bf, :], wb[:, : n_bf * 128]
                            )
                        if n_bf < SUBC:
                            wsv = wst[:, :].rearrange("p (j c) -> p j c", c=128)
                            d0 = j0 + n_bf - JS
                            nc.sync.dma_start_transpose(
                                wsv[:, d0 : d0 + SUBC - n_bf, :],
                                wb[:, n_bf * 128 :],
                            )
                    # cast the fp8 slice on ACT (Copy table, shared w/ flush)
                    nc.scalar.activation(wt8[oc][:, :], wst[:, :], AFT.Copy)

                emit_exps(range(2))
                dequant_oc(0)
                emit_exps(range(2, OC))
                for oc in range(1, OC):
                    dequant_oc(oc)

            # ---------- GEMM ----------
            xt_r = xt_d.rearrange("(a b p) m -> p a b m", p=128, b=XJ)
            x8_r = x8_d.rearrange("(pr pl p) m -> p pr pl m", p=128, pl=2)
            wto = [
                wt[oc][:, :].rearrange("p (j c) -> p j c", c=128) for oc in range(OC)
            ]
            wt8o = [
                wt8[oc][:, :].rearrange("p (pr pl c) -> p pr pl c", pl=2, c=128)
                for oc in range(OC)
            ]

            def load_x(mc):
                xts = []
                for a in range(XT_N):
                    xtile = mp.tile(
                        [128, XJ, 512], bf16, tag="xt", bufs=XB, name="xt"
                    )
                    nc.sync.dma_start(
                        xtile[:, :, :],
                        xt_r[:, a, :, mc * 512 : (mc + 1) * 512],
                    )
                    xts.append(xtile)
                x8t = mp.tile([128, kf, 2, 512], fp8, tag="x8t", bufs=2, name="x8t")
                nc.sync.dma_start(
                    x8t[:, :, :, :], x8_r[:, :, :, mc * 512 : (mc + 1) * 512]
                )
                return xts, x8t

            def mm_tile(ps, xts, x8t, oc):
                for j in range(JS):
                    nc.tensor.matmul(
                        ps[:, :],
                        wto[oc][:, j, :],
                        xts[j // XJ][:, j % XJ, :],
                        start=(j == 0),
                        stop=False,
                    )
                for pr in range(kf):
                    nc.tensor.matmul(
                        ps[:, :],
                        wt8o[oc][:, pr],
                        x8t[:, pr],
                        start=False,
                        stop=(pr == kf - 1),
                        perf_mode=PM.DoubleRow,
                    )

            def flush(ps, oc, mc):
                ob = mp.tile([128, 512], f32, tag="ob", bufs=2, name="ob")
                nc.scalar.activation(ob[:, :], ps[:, :], AFT.Copy)
                nc.sync.dma_start(
                    out[oc * 128 : (oc + 1) * 128, mc * 512 : (mc + 1) * 512],
                    ob[:, :],
                )

            # mc0+mc1: interleaved oc-major (PE gets 2 mcs of work per
            # dequanted out-chunk while prep streams)
            xts0, x8t0 = load_x(0)
            xts1, x8t1 = load_x(1)
            for oc in range(OC):
                for xts_i, x8t_i, mci in ((xts0, x8t0, 0), (xts1, x8t1, 1)):
                    ps = psp.tile([128, 512], f32, tag="ps", bufs=8, name="ps")
                    mm_tile(ps, xts_i, x8t_i, oc)
                    flush(ps, oc, mci)

            # mc2..: j-major, four passes of 4 out-chunks (finer availability)
            for mc in range(2, MC):
                xts, x8t = load_x(mc)
                for quarter in range(4):
                    pss = [
                        psp.tile([128, 512], f32, tag="ps", bufs=8, name="ps")
                        for _ in range(4)
                    ]
                    ocs = list(range(quarter * 4, quarter * 4 + 4))
                    for j in range(JS):
                        for i, oc in enumerate(ocs):
                            nc.tensor.matmul(
                                pss[i][:, :],
                                wto[oc][:, j, :],
                                xts[j // XJ][:, j % XJ, :],
                                start=(j == 0),
                                stop=False,
                            )
                    for pr in range(kf):
                        for i, oc in enumerate(ocs):
                            nc.tensor.matmul(
                                pss[i][:, :],
                                wt8o[oc][:, pr],
                                x8t[:, pr],
                                start=False,
                                stop=(pr == kf - 1),
                                perf_mode=PM.DoubleRow,
                            )
                    for i, oc in enumerate(ocs):
                        flush(pss[i], oc, mc)

    nc.finalize()
    return nc


def get_nc(M, N, K, coefs, n_cores):
    key = (M, N, K, coefs, n_cores, KF, KW)
    if key not in _BUILD_CACHE:
        _BUILD_CACHE[key] = _build(M, N, K, coefs, n_cores)
    return _BUILD_CACHE[key]


def make_in_maps(x, codebook, scale, indexes):
    import ml_dtypes

    x = np.asarray(x, dtype=np.float32)
    codebook = np.asarray(codebook, dtype=np.float32)
    scale = np.asarray(scale, dtype=np.float32)
    indexes = np.asarray(indexes, dtype=np.int32)

    Bx, Sx, INx = x.shape
    OUTx = indexes.shape[0]
    M = Bx * Sx
    coefs = _fit_cubic(codebook)
    KS = INx - 2 * KF * 128

    xT = x.reshape(M, INx).T  # [K, M] view
    xt_bf = np.ascontiguousarray(xT[:KS].astype(ml_dtypes.bfloat16))
    x8 = np.ascontiguousarray(
        (xT[KS:] * np.float32(1.0 / KW)).astype(ml_dtypes.float8_e4m3)
    )
    cb = np.asarray(codebook, np.float64).reshape(-1)
    cbn = cb / np.clip(np.abs(cb).max(), 1e-8, None)
    alpha, beta = float(cbn[2]), float(cbn[3])
    BoA = (3 * alpha - beta) / (2 * (beta - alpha))
    tenc = (
        indexes.reshape(OUTx, INx).astype(np.float32) - 1.5 - np.float32(BoA)
    ).astype(ml_dtypes.bfloat16)
    scl2 = np.ascontiguousarray(scale.reshape(OUTx, INx // GS))

    n_shard = OUTx // N_CORES
    in_maps = []
    for c in range(N_CORES):
        in_maps.append(
            {
                "xt": xt_bf,
                "x8": x8,
                "idx": np.ascontiguousarray(tenc[c * n_shard : (c + 1) * n_shard]),
                "scl": scl2[c * n_shard : (c + 1) * n_shard],
            }
        )
    return in_maps, coefs, (Bx, Sx, INx, OUTx, M, n_shard)


def kernel(x, codebook, scale, indexes):
    from concourse import bass_utils

    in_maps, coefs, (Bx, Sx, INx, OUTx, M, n_shard) = make_in_maps(
        x, codebook, scale, indexes
    )
    nc = get_nc(M, n_shard, INx, coefs, N_CORES)
    res = bass_utils.run_bass_kernel_spmd(nc, in_maps, core_ids=list(range(N_CORES)))
    out_t = np.concatenate(
        [res.results[c]["out"] for c in range(N_CORES)], axis=0
    )  # [OUT, M]
    return np.ascontiguousarray(out_t.T).reshape(Bx, Sx, OUTx)
